# revision 36
# baseline (speedup 1.0000x reference)
"""Top-1 (Switch) MoE layer on 8 Trainium2 NeuronCores — expert parallelism.

Sharding strategy:
  - Each core e owns expert e's weights (wi[e], wo[e]) — expert parallel.
  - Host computes the dispatch (router argmax) and all-to-alls the tokens:
    core e receives the tokens routed to expert e (padded to a fixed
    capacity), prepacked into the exact SBUF layout (128-partition-major,
    contiguous per partition) so every DMA is a long linear run.
  - Router weights are replicated; each core re-computes router logits
    on-device for (a) its contiguous 256-token shard (for the full
    router_logits / expert_index outputs) in plain fp32 so argmax matches
    the reference bit-for-bit, and (b) its gathered tokens (for the top-1
    probability that scales the FFN output).
  - FFN matmuls run in fp16 (fast weight load + half the HBM traffic);
    the router stays fp32. FFN1/FFN2 interleave per d_ff chunk so wo
    streams in behind wi; the router runs at the tail where the PE would
    otherwise idle while outputs drain.
  - Host scatters each core's [capacity, d_model] result back to token
    order (pure data movement) and concatenates the shard outputs.
"""

import numpy as np

NUM_EXPERTS = 8
D_MODEL = 512
D_FF = 2048
BATCH, SEQ = 1, 2048
N_CORES = 8
SHARD = SEQ // N_CORES          # 256 router tokens per core
KC = D_MODEL // 128             # 4 contraction chunks of d_model
NF = D_FF // 128                # 16 chunks of d_ff
NG = 2                          # d_ff chunk groups (stream granularity)
NFG = NF // NG                  # f-chunks per group
NS = SHARD // 128               # 2 shard tiles

DTYPE = "fp16"                  # "fp16" | "bf16" | "f32r"
WARMUP_MM = 18                 # dummy PE matmuls to release the HAM clock gate
FFN2_LAG = 1                    # FFN2 trails FFN1 by this many f-rounds (wo stream slack)

_CACHE = {}
LAST_RESULTS = None
PROFILE = False
TRACE_CORES = None


def _np_ff():
    if DTYPE == "fp16":
        return np.float16
    if DTYPE == "bf16":
        import ml_dtypes

        return ml_dtypes.bfloat16
    return np.float32


def _build(c_pad, dtype_name):
    import concourse.bacc as bacc
    import concourse.bass as bass
    import concourse.tile as tile
    from concourse import mybir

    f32 = mybir.dt.float32
    i32 = mybir.dt.int32
    ff = {
        "fp16": mybir.dt.float16,
        "bf16": mybir.dt.bfloat16,
        "f32r": mybir.dt.float32r,
    }[dtype_name]
    AF = mybir.ActivationFunctionType
    ALU = mybir.AluOpType
    AX = mybir.AxisListType
    ts = bass.ts

    m_sizes = [min(128, c_pad - m * 128) for m in range(-(-c_pad // 128))]
    nm = len(m_sizes)
    nc = bacc.Bacc(None, target_bir_lowering=False)

    # All inputs prepacked on host to [128, ...] partition-major layouts.
    xg_t = nc.dram_tensor("xg_t", [128, KC, c_pad], ff, kind="ExternalInput")
    xs_t = nc.dram_tensor("xs_t", [128, KC, SHARD], f32, kind="ExternalInput")
    wr = nc.dram_tensor("wr", [128, KC, NUM_EXPERTS], f32, kind="ExternalInput")
    wr_r = nc.dram_tensor("wr_r", [128, KC, NUM_EXPERTS], ff, kind="ExternalInput")
    wi = nc.dram_tensor("wi", [NG, 128, KC, NFG * 128], ff, kind="ExternalInput")
    wo = nc.dram_tensor("wo", [NG, 128, NFG, D_MODEL], ff, kind="ExternalInput")

    y_out = nc.dram_tensor("y_out", [128, nm, D_MODEL], ff, kind="ExternalOutput")
    lg_out = nc.dram_tensor("lg_out", [128, NS, NUM_EXPERTS], f32, kind="ExternalOutput")
    ix_out = nc.dram_tensor("ix_out", [128, NS], i32, kind="ExternalOutput")

    with tile.TileContext(nc) as tc:
        with (
            tc.tile_pool(name="weights", bufs=1) as wpool,
            tc.tile_pool(name="acts", bufs=1) as apool,
            tc.tile_pool(name="hbuf", bufs=3) as hpool,
            tc.tile_pool(name="small", bufs=2) as spool,
            tc.tile_pool(name="psum_r", bufs=3, space="PSUM") as prpool,
            tc.tile_pool(name="psum_h", bufs=2, space="PSUM") as phpool,
            tc.tile_pool(name="psum_y", bufs=1, space="PSUM") as pypool,
        ):
            # ---- PE warm-up: dummy matmuls on scratch while inputs stream,
            # so the HAM clock-gate releases before the real FFN begins.
            # (Reuses the ph psum slots — PE-only, no cross-engine consumers.)
            if WARMUP_MM:
                scr = wpool.tile([128, 512], ff, tag="scr")
                nc.gpsimd.memset(scr[:, :], 0.0)
                for _ in range(WARMUP_MM):
                    pw = phpool.tile([128, 512], f32, tag="ph", name="pw")
                    nc.tensor.matmul(
                        pw[:, :], scr[:, :128], scr[:, :], start=True, stop=True
                    )
            # Two HWDGE rings: sync carries everything the PE needs early
            # (wrr, xg, then wi/wo 1MB chunks in consumption order — 8KB
            # per-partition runs keep the ring at full rate); scalar carries
            # the tail-router inputs and the outputs.
            wi_g = [None] * NG
            wo_g = [None] * NG

            def _wi_tile(g):
                t = wpool.tile([128, KC, NFG * 128], ff, tag=f"wi{g}", name=f"wi{g}")
                wi_g[g] = t
                return t

            def _wo_tile(g):
                t = wpool.tile([128, NFG, D_MODEL], ff, tag=f"wo{g}", name=f"wo{g}")
                wo_g[g] = t
                return t

            # sync ring starts ~3us earlier than scalar's — everything the PE
            # needs early goes there, in consumption order.
            wrr_sb = wpool.tile([128, KC, NUM_EXPERTS], ff, tag="wrr")
            nc.sync.dma_start(out=wrr_sb[:], in_=wr_r[:])
            xg_sb = apool.tile([128, KC, c_pad], ff, tag="xg")
            nc.sync.dma_start(out=xg_sb[:], in_=xg_t[:])
            nc.sync.dma_start(out=_wi_tile(0)[:], in_=wi[0])
            nc.sync.dma_start(out=_wo_tile(0)[:], in_=wo[0])
            nc.sync.dma_start(out=_wi_tile(1)[:], in_=wi[1])
            nc.sync.dma_start(out=_wo_tile(1)[:], in_=wo[1])
            xs_sb = apool.tile([128, KC, SHARD], f32, tag="xs")
            nc.scalar.dma_start(out=xs_sb[:], in_=xs_t[:])
            wr_sb = wpool.tile([128, KC, NUM_EXPERTS], f32, tag="wr")
            nc.scalar.dma_start(out=wr_sb[:], in_=wr[:])

            # descending iota 7..0, used to pick the FIRST max (argmax)
            iota_i = wpool.tile([128, NUM_EXPERTS], i32, tag="iota_i")
            nc.gpsimd.iota(
                iota_i[:, :], pattern=[[-1, NUM_EXPERTS]], base=7, channel_multiplier=0
            )
            iota_f = wpool.tile([128, NUM_EXPERTS], f32, tag="iota_f")
            nc.vector.tensor_copy(iota_f[:, :], iota_i[:, :])

            # ---- top-1 probability for the gathered tokens (early: only
            # needs xg + wrr, and prob unblocks the y tail) ----
            prob_sb = apool.tile([128, nm], f32, tag="prob")
            for m in range(nm):
                sz = m_sizes[m]
                pg = prpool.tile([128, NUM_EXPERTS], f32, tag="pr")
                for c in range(KC):
                    nc.tensor.matmul(
                        pg[:sz, :],
                        xg_sb[:, c, m * 128 : m * 128 + sz],
                        wrr_sb[:, c, :],
                        start=(c == 0),
                        stop=(c == KC - 1),
                    )
                gmax = spool.tile([128, 1], f32, tag="gmax")
                nc.vector.tensor_reduce(gmax[:sz, :], pg[:sz, :], axis=AX.X, op=ALU.max)
                ls = spool.tile([128, NUM_EXPERTS], f32, tag="ls")
                nc.vector.tensor_scalar(ls[:sz, :], pg[:sz, :], gmax[:sz, :], None, ALU.subtract)
                ex = spool.tile([128, NUM_EXPERTS], f32, tag="ex")
                esum = spool.tile([128, 1], f32, tag="esum")
                nc.scalar.activation(ex[:sz, :], ls[:sz, :], AF.Exp, accum_out=esum[:sz, :])
                nc.vector.reciprocal(prob_sb[:sz, m : m + 1], esum[:sz, :])

            # ---- FFN1 + FFN2 interleaved per d_ff chunk; FFN2 trails by
            # FFN2_LAG rounds so the wo stream has slack ----
            py_tiles = [
                pypool.tile([128, D_MODEL], f32, tag=f"py{m}", name=f"py{m}")
                for m in range(nm)
            ]
            hf_tiles = [None] * NF

            def _ffn2(f):
                g, fl = divmod(f, NFG)
                for m in range(nm):
                    sz = m_sizes[m]
                    nc.tensor.matmul(
                        py_tiles[m][:sz, :],
                        hf_tiles[f][:, m * 128 : m * 128 + sz],
                        wo_g[g][:, fl, :],
                        start=(f == 0),
                        stop=(f == NF - 1),
                    )

            for f in range(NF):
                g, fl = divmod(f, NFG)
                ph = phpool.tile([128, c_pad], f32, tag="ph")
                for c in range(KC):
                    nc.tensor.matmul(
                        ph[:, :],
                        wi_g[g][:, c, ts(fl, 128)],
                        xg_sb[:, c, :],
                        start=(c == 0),
                        stop=(c == KC - 1),
                    )
                hf = hpool.tile([128, c_pad], ff, tag="hf")
                hf_tiles[f] = hf
                nc.vector.tensor_scalar_max(hf[:, :], ph[:, :], 0.0)
                if f >= FFN2_LAG:
                    _ffn2(f - FFN2_LAG)
            for f in range(NF - FFN2_LAG, NF):
                _ffn2(f)

            # ---- scale (DVE/ACT alternating) + store y on both rings ----
            y_sb = apool.tile([128, nm, D_MODEL], ff, tag="y")
            for m in range(nm):
                sz = m_sizes[m]
                if m % 2 == 0:
                    nc.vector.tensor_scalar(
                        y_sb[:sz, m, :], py_tiles[m][:sz, :], prob_sb[:sz, m : m + 1],
                        None, ALU.mult,
                    )
                else:
                    nc.scalar.activation(
                        y_sb[:sz, m, :], py_tiles[m][:sz, :], AF.Copy,
                        scale=prob_sb[:sz, m : m + 1],
                    )
            for m in range(nm):
                sz = m_sizes[m]
                eng = nc.sync if m % 2 == 0 else nc.scalar
                eng.dma_start(out=y_out[:sz, m, :], in_=y_sb[:sz, m, :])

            # ---- router on the contiguous shard (fp32, exact argmax, tail) ----
            lg_sb = apool.tile([128, NS, NUM_EXPERTS], f32, tag="lg")
            ix_sb = apool.tile([128, NS], i32, tag="ix")
            for m in range(NS):
                pl = prpool.tile([128, NUM_EXPERTS], f32, tag="pr")
                for c in range(KC):
                    nc.tensor.matmul(
                        pl[:, :],
                        xs_sb[:, c, ts(m, 128)],
                        wr_sb[:, c, :],
                        start=(c == 0),
                        stop=(c == KC - 1),
                    )
                nc.vector.tensor_copy(lg_sb[:, m, :], pl[:, :])
                lmax = spool.tile([128, 1], f32, tag="lmax")
                nc.vector.tensor_reduce(lmax[:, :], pl[:, :], axis=AX.X, op=ALU.max)
                eq = spool.tile([128, NUM_EXPERTS], f32, tag="eq")
                nc.vector.tensor_scalar(eq[:, :], pl[:, :], lmax[:, :], None, ALU.is_equal)
                nc.vector.tensor_mul(eq[:, :], eq[:, :], iota_f[:, :])
                m7 = spool.tile([128, 1], f32, tag="m7")
                nc.vector.tensor_reduce(m7[:, :], eq[:, :], axis=AX.X, op=ALU.max)
                idxf = spool.tile([128, 1], f32, tag="idxf")
                nc.vector.tensor_scalar(
                    idxf[:, :], m7[:, :], -1.0, 7.0, ALU.mult, ALU.add
                )
                nc.vector.tensor_copy(ix_sb[:, m : m + 1], idxf[:, :])
            nc.scalar.dma_start(out=lg_out[:], in_=lg_sb[:])
            nc.scalar.dma_start(out=ix_out[:], in_=ix_sb[:])

    nc.compile()
    return nc


def _dispatch(x, wr):
    """Host-side all-to-all dispatch decision: token -> expert."""
    logits = x @ wr
    eidx = np.argmax(logits, axis=-1)
    counts = np.bincount(eidx, minlength=NUM_EXPERTS)
    c_pad = max(256, -(-int(counts.max()) // 128) * 128)
    return eidx, counts, c_pad


def _pack_k_major(a2d, inner):
    """[512, N] -> [128, KC, N] with row (c*128+p) at [p, c]."""
    return np.ascontiguousarray(a2d.reshape(KC, 128, inner).transpose(1, 0, 2))


def kernel(hidden_states, w_router, wi, wo):
    x = np.ascontiguousarray(
        np.asarray(hidden_states, dtype=np.float32).reshape(SEQ, D_MODEL)
    )
    wr = np.ascontiguousarray(np.asarray(w_router, dtype=np.float32))
    wi = np.asarray(wi, dtype=np.float32)
    wo = np.asarray(wo, dtype=np.float32)

    eidx, counts, c_pad = _dispatch(x, wr)

    key = (c_pad, DTYPE)
    nc = _CACHE.get(key)
    if nc is None:
        nc = _build(c_pad, DTYPE)
        _CACHE[key] = nc

    np_ff = _np_ff()
    wr_p = _pack_k_major(wr, NUM_EXPERTS)

    tok_lists = [np.nonzero(eidx == e)[0] for e in range(NUM_EXPERTS)]
    in_maps = []
    for e in range(NUM_EXPERTS):
        toks = tok_lists[e]
        xg = np.zeros((c_pad, D_MODEL), np.float32)
        xg[: len(toks)] = x[toks]
        # wi[e]: [512, 2048] -> [NG, 128, KC, 512] : [g, p, c, j] = wi[c*128+p, g*512+j]
        wi_p = np.ascontiguousarray(
            wi[e].reshape(KC, 128, NG, NFG * 128).transpose(2, 1, 0, 3)
        ).astype(np_ff)
        # wo[e]: [2048, 512] -> [NG, 128, NFG, 512] : [g, p, fl, d] = wo[(g*4+fl)*128+p, d]
        wo_p = np.ascontiguousarray(
            wo[e].reshape(NG, NFG, 128, D_MODEL).transpose(0, 2, 1, 3)
        ).astype(np_ff)
        in_maps.append(
            {
                "xg_t": _pack_k_major(xg.T, c_pad).astype(np_ff),
                "xs_t": _pack_k_major(
                    np.ascontiguousarray(x[e * SHARD : (e + 1) * SHARD].T), SHARD
                ),
                "wr": wr_p,
                "wr_r": wr_p.astype(np_ff),
                "wi": wi_p,
                "wo": wo_p,
            }
        )

    from concourse.bass_utils import run_bass_kernel_spmd

    global LAST_RESULTS
    res = run_bass_kernel_spmd(
        nc,
        in_maps,
        list(range(N_CORES)),
        trace=PROFILE,
        trace_cores=TRACE_CORES,
    )
    LAST_RESULTS = res

    out = np.empty((SEQ, D_MODEL), np.float32)
    lg_full = np.empty((SEQ, NUM_EXPERTS), np.float32)
    ix_full = np.empty((SEQ,), np.int32)
    for e in range(NUM_EXPERTS):
        r = res.results[e]
        toks = tok_lists[e]
        # y_out [128, nm, 512]: token j of the gathered order is [j % 128, j // 128]
        y = r["y_out"].transpose(1, 0, 2).reshape(-1, D_MODEL)
        out[toks] = y[: len(toks)]
        # lg_out [128, NS, 8] / ix_out [128, NS]: shard token t = m*128+p -> [p, m]
        lg_full[e * SHARD : (e + 1) * SHARD] = (
            r["lg_out"].transpose(1, 0, 2).reshape(SHARD, NUM_EXPERTS)
        )
        ix_full[e * SHARD : (e + 1) * SHARD] = r["ix_out"].T.reshape(SHARD)

    return (
        out.reshape(BATCH, SEQ, D_MODEL),
        (
            lg_full.reshape(BATCH, SEQ, NUM_EXPERTS),
            ix_full.reshape(BATCH, SEQ),
        ),
    )


# revision 37
# speedup vs baseline: 1.1788x; 1.1788x over previous
"""Top-1 (Switch) MoE layer on 8 Trainium2 NeuronCores — expert parallelism.

Sharding strategy:
  - Each core e owns expert e's weights (wi[e], wo[e]) — expert parallel.
  - Host computes the dispatch (router argmax) and all-to-alls the tokens:
    core e receives the tokens routed to expert e (padded to a fixed
    capacity), prepacked into the exact SBUF layout (128-partition-major,
    contiguous per partition) so every DMA is a long linear run.
  - Router weights are replicated; each core re-computes router logits
    on-device for (a) its contiguous 256-token shard (for the full
    router_logits / expert_index outputs) in plain fp32 so argmax matches
    the reference bit-for-bit, and (b) its gathered tokens (for the top-1
    probability that scales the FFN output).
  - FFN matmuls run in fp16 (fast weight load + half the HBM traffic);
    the router stays fp32. FFN1/FFN2 interleave per d_ff chunk so wo
    streams in behind wi; the router runs at the tail where the PE would
    otherwise idle while outputs drain.
  - Host scatters each core's [capacity, d_model] result back to token
    order (pure data movement) and concatenates the shard outputs.
"""

import numpy as np

NUM_EXPERTS = 8
D_MODEL = 512
D_FF = 2048
BATCH, SEQ = 1, 2048
N_CORES = 8
SHARD = SEQ // N_CORES          # 256 router tokens per core
KC = D_MODEL // 128             # 4 contraction chunks of d_model
NF = D_FF // 128                # 16 chunks of d_ff
NG = 2                          # d_ff chunk groups (stream granularity)
NFG = NF // NG                  # f-chunks per group
NS = SHARD // 128               # 2 shard tiles

DTYPE = "fp16"                  # "fp16" | "bf16" | "f32r"
WARMUP_MM = 18                 # dummy PE matmuls to release the HAM clock gate
FFN2_LAG = 2                    # FFN2 trails FFN1 by this many f-rounds (wo stream slack)

_CACHE = {}
LAST_RESULTS = None
PROFILE = False
TRACE_CORES = None


def _np_ff():
    if DTYPE == "fp16":
        return np.float16
    if DTYPE == "bf16":
        import ml_dtypes

        return ml_dtypes.bfloat16
    return np.float32


def _build(c_pad, dtype_name):
    import concourse.bacc as bacc
    import concourse.bass as bass
    import concourse.tile as tile
    from concourse import mybir

    f32 = mybir.dt.float32
    i32 = mybir.dt.int32
    ff = {
        "fp16": mybir.dt.float16,
        "bf16": mybir.dt.bfloat16,
        "f32r": mybir.dt.float32r,
    }[dtype_name]
    AF = mybir.ActivationFunctionType
    ALU = mybir.AluOpType
    AX = mybir.AxisListType
    ts = bass.ts

    m_sizes = [min(128, c_pad - m * 128) for m in range(-(-c_pad // 128))]
    nm = len(m_sizes)
    nc = bacc.Bacc(None, target_bir_lowering=False)

    # All inputs prepacked on host to [128, ...] partition-major layouts.
    xg_t = nc.dram_tensor("xg_t", [128, KC, c_pad], ff, kind="ExternalInput")
    xs_t = nc.dram_tensor("xs_t", [128, KC, SHARD], f32, kind="ExternalInput")
    wr = nc.dram_tensor("wr", [128, KC, NUM_EXPERTS], f32, kind="ExternalInput")
    wr_r = nc.dram_tensor("wr_r", [128, KC, NUM_EXPERTS], ff, kind="ExternalInput")
    wi = nc.dram_tensor("wi", [NG, 128, KC, NFG * 128], ff, kind="ExternalInput")
    wo = nc.dram_tensor("wo", [NG, 128, NFG, D_MODEL], ff, kind="ExternalInput")

    y_out = nc.dram_tensor("y_out", [128, nm, D_MODEL], ff, kind="ExternalOutput")
    lg_out = nc.dram_tensor("lg_out", [128, NS, NUM_EXPERTS], f32, kind="ExternalOutput")
    ix_out = nc.dram_tensor("ix_out", [128, NS], i32, kind="ExternalOutput")

    with tile.TileContext(nc) as tc:
        with (
            tc.tile_pool(name="weights", bufs=1) as wpool,
            tc.tile_pool(name="acts", bufs=1) as apool,
            tc.tile_pool(name="hbuf", bufs=3) as hpool,
            tc.tile_pool(name="small", bufs=2) as spool,
            tc.tile_pool(name="psum_r", bufs=3, space="PSUM") as prpool,
            tc.tile_pool(name="psum_h", bufs=2, space="PSUM") as phpool,
            tc.tile_pool(name="psum_y", bufs=1, space="PSUM") as pypool,
        ):
            # ---- PE warm-up: dummy matmuls on scratch while inputs stream,
            # so the HAM clock-gate releases before the real FFN begins.
            # (Reuses the ph psum slots — PE-only, no cross-engine consumers.)
            if WARMUP_MM:
                scr = wpool.tile([128, 512], ff, tag="scr")
                nc.gpsimd.memset(scr[:, :], 0.0)
                for _ in range(WARMUP_MM):
                    pw = phpool.tile([128, 512], f32, tag="ph", name="pw")
                    nc.tensor.matmul(
                        pw[:, :], scr[:, :128], scr[:, :], start=True, stop=True
                    )
            # Two HWDGE rings: sync carries everything the PE needs early
            # (wrr, xg, then wi/wo 1MB chunks in consumption order — 8KB
            # per-partition runs keep the ring at full rate); scalar carries
            # the tail-router inputs and the outputs.
            wi_g = [None] * NG
            wo_g = [None] * NG

            def _wi_tile(g):
                t = wpool.tile([128, KC, NFG * 128], ff, tag=f"wi{g}", name=f"wi{g}")
                wi_g[g] = t
                return t

            def _wo_tile(g):
                t = wpool.tile([128, NFG, D_MODEL], ff, tag=f"wo{g}", name=f"wo{g}")
                wo_g[g] = t
                return t

            # sync ring starts ~3us earlier than scalar's — everything the PE
            # needs early goes there, in consumption order.
            wrr_sb = wpool.tile([128, KC, NUM_EXPERTS], ff, tag="wrr")
            nc.sync.dma_start(out=wrr_sb[:], in_=wr_r[:])
            xg_sb = apool.tile([128, KC, c_pad], ff, tag="xg")
            nc.sync.dma_start(out=xg_sb[:], in_=xg_t[:])
            nc.sync.dma_start(out=_wi_tile(0)[:], in_=wi[0])
            nc.sync.dma_start(out=_wo_tile(0)[:], in_=wo[0])
            nc.sync.dma_start(out=_wi_tile(1)[:], in_=wi[1])
            nc.sync.dma_start(out=_wo_tile(1)[:], in_=wo[1])
            xs_sb = apool.tile([128, KC, SHARD], f32, tag="xs")
            nc.scalar.dma_start(out=xs_sb[:], in_=xs_t[:])
            wr_sb = wpool.tile([128, KC, NUM_EXPERTS], f32, tag="wr")
            nc.scalar.dma_start(out=wr_sb[:], in_=wr[:])

            # descending iota 7..0, used to pick the FIRST max (argmax)
            iota_i = wpool.tile([128, NUM_EXPERTS], i32, tag="iota_i")
            nc.gpsimd.iota(
                iota_i[:, :], pattern=[[-1, NUM_EXPERTS]], base=7, channel_multiplier=0
            )
            iota_f = wpool.tile([128, NUM_EXPERTS], f32, tag="iota_f")
            nc.vector.tensor_copy(iota_f[:, :], iota_i[:, :])

            # ---- top-1 probability for the gathered tokens (early: only
            # needs xg + wrr, and prob unblocks the y tail) ----
            prob_sb = apool.tile([128, nm], f32, tag="prob")
            for m in range(nm):
                sz = m_sizes[m]
                pg = prpool.tile([128, NUM_EXPERTS], f32, tag="pr")
                for c in range(KC):
                    nc.tensor.matmul(
                        pg[:sz, :],
                        xg_sb[:, c, m * 128 : m * 128 + sz],
                        wrr_sb[:, c, :],
                        start=(c == 0),
                        stop=(c == KC - 1),
                    )
                gmax = spool.tile([128, 1], f32, tag="gmax")
                nc.vector.tensor_reduce(gmax[:sz, :], pg[:sz, :], axis=AX.X, op=ALU.max)
                ls = spool.tile([128, NUM_EXPERTS], f32, tag="ls")
                nc.vector.tensor_scalar(ls[:sz, :], pg[:sz, :], gmax[:sz, :], None, ALU.subtract)
                ex = spool.tile([128, NUM_EXPERTS], f32, tag="ex")
                esum = spool.tile([128, 1], f32, tag="esum")
                nc.scalar.activation(ex[:sz, :], ls[:sz, :], AF.Exp, accum_out=esum[:sz, :])
                nc.vector.reciprocal(prob_sb[:sz, m : m + 1], esum[:sz, :])

            # ---- FFN1 + FFN2 interleaved per d_ff chunk; FFN2 trails by
            # FFN2_LAG rounds so the wo stream has slack ----
            py_tiles = [
                pypool.tile([128, D_MODEL], f32, tag=f"py{m}", name=f"py{m}")
                for m in range(nm)
            ]
            hf_tiles = [None] * NF

            def _ffn2(f):
                g, fl = divmod(f, NFG)
                for m in range(nm):
                    sz = m_sizes[m]
                    nc.tensor.matmul(
                        py_tiles[m][:sz, :],
                        hf_tiles[f][:, m * 128 : m * 128 + sz],
                        wo_g[g][:, fl, :],
                        start=(f == 0),
                        stop=(f == NF - 1),
                    )

            for f in range(NF):
                g, fl = divmod(f, NFG)
                ph = phpool.tile([128, c_pad], f32, tag="ph")
                for c in range(KC):
                    nc.tensor.matmul(
                        ph[:, :],
                        wi_g[g][:, c, ts(fl, 128)],
                        xg_sb[:, c, :],
                        start=(c == 0),
                        stop=(c == KC - 1),
                    )
                hf = hpool.tile([128, c_pad], ff, tag="hf")
                hf_tiles[f] = hf
                nc.vector.tensor_scalar_max(hf[:, :], ph[:, :], 0.0)
                if f >= FFN2_LAG:
                    _ffn2(f - FFN2_LAG)
            for f in range(NF - FFN2_LAG, NF):
                _ffn2(f)

            # ---- scale (DVE/ACT alternating) + store y on both rings ----
            y_sb = apool.tile([128, nm, D_MODEL], ff, tag="y")
            for m in range(nm):
                sz = m_sizes[m]
                if m % 2 == 0:
                    nc.vector.tensor_scalar(
                        y_sb[:sz, m, :], py_tiles[m][:sz, :], prob_sb[:sz, m : m + 1],
                        None, ALU.mult,
                    )
                else:
                    nc.scalar.activation(
                        y_sb[:sz, m, :], py_tiles[m][:sz, :], AF.Copy,
                        scale=prob_sb[:sz, m : m + 1],
                    )
            for m in range(nm):
                sz = m_sizes[m]
                eng = nc.sync if m % 2 == 0 else nc.scalar
                eng.dma_start(out=y_out[:sz, m, :], in_=y_sb[:sz, m, :])

            # ---- router on the contiguous shard (fp32, exact argmax, tail) ----
            lg_sb = apool.tile([128, NS, NUM_EXPERTS], f32, tag="lg")
            ix_sb = apool.tile([128, NS], i32, tag="ix")
            for m in range(NS):
                pl = prpool.tile([128, NUM_EXPERTS], f32, tag="pr")
                for c in range(KC):
                    nc.tensor.matmul(
                        pl[:, :],
                        xs_sb[:, c, ts(m, 128)],
                        wr_sb[:, c, :],
                        start=(c == 0),
                        stop=(c == KC - 1),
                    )
                nc.vector.tensor_copy(lg_sb[:, m, :], pl[:, :])
                lmax = spool.tile([128, 1], f32, tag="lmax")
                nc.vector.tensor_reduce(lmax[:, :], pl[:, :], axis=AX.X, op=ALU.max)
                eq = spool.tile([128, NUM_EXPERTS], f32, tag="eq")
                nc.vector.tensor_scalar(eq[:, :], pl[:, :], lmax[:, :], None, ALU.is_equal)
                nc.vector.tensor_mul(eq[:, :], eq[:, :], iota_f[:, :])
                m7 = spool.tile([128, 1], f32, tag="m7")
                nc.vector.tensor_reduce(m7[:, :], eq[:, :], axis=AX.X, op=ALU.max)
                idxf = spool.tile([128, 1], f32, tag="idxf")
                nc.vector.tensor_scalar(
                    idxf[:, :], m7[:, :], -1.0, 7.0, ALU.mult, ALU.add
                )
                nc.vector.tensor_copy(ix_sb[:, m : m + 1], idxf[:, :])
            nc.scalar.dma_start(out=lg_out[:], in_=lg_sb[:])
            nc.scalar.dma_start(out=ix_out[:], in_=ix_sb[:])

    nc.compile()
    return nc


def _dispatch(x, wr):
    """Host-side all-to-all dispatch decision: token -> expert."""
    logits = x @ wr
    eidx = np.argmax(logits, axis=-1)
    counts = np.bincount(eidx, minlength=NUM_EXPERTS)
    c_pad = max(256, -(-int(counts.max()) // 128) * 128)
    return eidx, counts, c_pad


def _pack_k_major(a2d, inner):
    """[512, N] -> [128, KC, N] with row (c*128+p) at [p, c]."""
    return np.ascontiguousarray(a2d.reshape(KC, 128, inner).transpose(1, 0, 2))


def kernel(hidden_states, w_router, wi, wo):
    x = np.ascontiguousarray(
        np.asarray(hidden_states, dtype=np.float32).reshape(SEQ, D_MODEL)
    )
    wr = np.ascontiguousarray(np.asarray(w_router, dtype=np.float32))
    wi = np.asarray(wi, dtype=np.float32)
    wo = np.asarray(wo, dtype=np.float32)

    eidx, counts, c_pad = _dispatch(x, wr)

    key = (c_pad, DTYPE)
    nc = _CACHE.get(key)
    if nc is None:
        nc = _build(c_pad, DTYPE)
        _CACHE[key] = nc

    np_ff = _np_ff()
    wr_p = _pack_k_major(wr, NUM_EXPERTS)

    tok_lists = [np.nonzero(eidx == e)[0] for e in range(NUM_EXPERTS)]
    in_maps = []
    for e in range(NUM_EXPERTS):
        toks = tok_lists[e]
        xg = np.zeros((c_pad, D_MODEL), np.float32)
        xg[: len(toks)] = x[toks]
        # wi[e]: [512, 2048] -> [NG, 128, KC, 512] : [g, p, c, j] = wi[c*128+p, g*512+j]
        wi_p = np.ascontiguousarray(
            wi[e].reshape(KC, 128, NG, NFG * 128).transpose(2, 1, 0, 3)
        ).astype(np_ff)
        # wo[e]: [2048, 512] -> [NG, 128, NFG, 512] : [g, p, fl, d] = wo[(g*4+fl)*128+p, d]
        wo_p = np.ascontiguousarray(
            wo[e].reshape(NG, NFG, 128, D_MODEL).transpose(0, 2, 1, 3)
        ).astype(np_ff)
        in_maps.append(
            {
                "xg_t": _pack_k_major(xg.T, c_pad).astype(np_ff),
                "xs_t": _pack_k_major(
                    np.ascontiguousarray(x[e * SHARD : (e + 1) * SHARD].T), SHARD
                ),
                "wr": wr_p,
                "wr_r": wr_p.astype(np_ff),
                "wi": wi_p,
                "wo": wo_p,
            }
        )

    from concourse.bass_utils import run_bass_kernel_spmd

    global LAST_RESULTS
    res = run_bass_kernel_spmd(
        nc,
        in_maps,
        list(range(N_CORES)),
        trace=PROFILE,
        trace_cores=TRACE_CORES,
    )
    LAST_RESULTS = res

    out = np.empty((SEQ, D_MODEL), np.float32)
    lg_full = np.empty((SEQ, NUM_EXPERTS), np.float32)
    ix_full = np.empty((SEQ,), np.int32)
    for e in range(NUM_EXPERTS):
        r = res.results[e]
        toks = tok_lists[e]
        # y_out [128, nm, 512]: token j of the gathered order is [j % 128, j // 128]
        y = r["y_out"].transpose(1, 0, 2).reshape(-1, D_MODEL)
        out[toks] = y[: len(toks)]
        # lg_out [128, NS, 8] / ix_out [128, NS]: shard token t = m*128+p -> [p, m]
        lg_full[e * SHARD : (e + 1) * SHARD] = (
            r["lg_out"].transpose(1, 0, 2).reshape(SHARD, NUM_EXPERTS)
        )
        ix_full[e * SHARD : (e + 1) * SHARD] = r["ix_out"].T.reshape(SHARD)

    return (
        out.reshape(BATCH, SEQ, D_MODEL),
        (
            lg_full.reshape(BATCH, SEQ, NUM_EXPERTS),
            ix_full.reshape(BATCH, SEQ),
        ),
    )
